# revision 8
# baseline (speedup 1.0000x reference)
"""Trainium2 Bass kernel for BasicMultiTokenPrediction.

Strategy (8 NeuronCores, SPMD, four pipelined launches):
  The module = 3 sequential transformer layers (cheap compute, 16MB of
  weights each) + a dominant vocab unembed GEMM per layer (86% of FLOPs).
  Layer k+1 only needs layer k's hidden rows, while unembed k needs ALL
  rows of h_k but is vocab-sharded. Splitting the work as
      launch 1:  layer 0                       (weight-DMA bound, ~46us)
      launch 2:  layer 1 + unembed(h_0)        (PE/DMA balanced, ~105us)
      launch 3:  layer 2 + unembed(h_1)        (PE/DMA balanced, ~105us)
      launch 4:  unembed(h_2)                  (PE bound, ~90us)
  lets each unembed phase hide the NEXT layer's 16MB weight stream under
  its ~88us of matmuls, instead of serializing all 48MB of weight DMA
  before/after the unembed as the 2-launch split does. The h rows are
  gathered across cores on the host between launches (1MB, free in HW
  time). Logits are stored bf16 (rel-err budget 2e-2; bf16 adds <4e-3)
  which halves the unembed's output traffic.

  Per launch: stage-A data-parallel over the 506 rows (64/core),
  activations kept transposed [feature-on-partition, row-on-free];
  unembed tensor-parallel over embed rows (6400-wide vocab shard/core,
  V padded 50257->51200); lhsT = gathered h (512 rows incl. pad), rhs =
  embed shard streamed through SBUF; 8 accumulating matmuls per 128x512
  PSUM tile. All GEMMs bf16 (fp32 accumulate), norms/residuals fp32.

Host side: shards/transposes/casts inputs, gathers h between launches,
concatenates vocab shards at the end. Zero biases / unit norm scales (the
values this module is specified with) skip their device ops entirely;
rms_w is always folded into the proj weights on the host (exact).
"""
import sys

try:
    import concourse.bass  # noqa: F401  (already on path in this env?)
except ImportError:
    sys.path.insert(0, "/opt/trn_rl_repo")

import numpy as np
import ml_dtypes
import concourse.bass as bass
import concourse.mybir as mybir
import concourse.tile as tile
from concourse import bacc
from concourse import bass2jax

BF16 = mybir.dt.bfloat16
F32 = mybir.dt.float32
BF16_NP = ml_dtypes.bfloat16

B, T, D, K, V, DFF = 2, 256, 1024, 3, 50257, 2048
P = T - K            # 253 starting positions
ROWS = B * P         # 506 independent rows
NCORES = 8
VS = 6400            # per-core vocab shard (8*6400 = 51200 >= 50257)
VPAD = VS * NCORES
DC = D // 128        # 8 contraction chunks of 128
RPC = 64             # stage-A rows per core (8*64 = 512 >= 506)
ROWSG = NCORES * RPC  # gathered rows incl. pad
NRT = ROWSG // 128   # 4 full row tiles for the unembed
FC_PROJ = (2 * D) // 128
OC = D // 128
EPS_LN = 1e-5
EPS_RMS = 1e-6

V_BLOCKS = [(j * 512, 512) for j in range(VS // 512)]
if VS % 512:
    V_BLOCKS.append((VS - VS % 512, VS % 512))
NBLK = len(V_BLOCKS)


# ===================================================== shared layer body ====

def declare_layer_inputs(nc, flags):
    aps = {
        "wproj": nc.dram_tensor("wproj", [128, FC_PROJ, D], BF16,
                                kind="ExternalInput").ap(),
        "wv": nc.dram_tensor("wv", [128, OC, D], BF16,
                             kind="ExternalInput").ap(),
        "wao": nc.dram_tensor("wao", [128, OC, D], BF16,
                              kind="ExternalInput").ap(),
        "wf1": nc.dram_tensor("wf1", [128, OC, DFF], BF16,
                              kind="ExternalInput").ap(),
        "wf2": nc.dram_tensor("wf2", [128, FC_PROJ, D], BF16,
                              kind="ExternalInput").ap(),
    }
    for nm, width in (("pbias", D), ("vbias", D), ("aobias", D),
                      ("f1bias", DFF), ("f2bias", D)):
        if flags.get(nm):
            aps[nm] = nc.dram_tensor(nm, [width], F32,
                                     kind="ExternalInput").ap()
    for nm in ("ln1", "ln2"):
        if flags.get(nm):
            aps[nm + "w"] = nc.dram_tensor(nm + "w", [D], F32,
                                           kind="ExternalInput").ap()
            aps[nm + "b"] = nc.dram_tensor(nm + "b", [D], F32,
                                           kind="ExternalInput").ap()
    return aps


class LayerOps:
    """Per-launch pools + helper ops for one transformer layer on 64 rows."""

    def __init__(self, nc, tc, flags, waps, stack, psum_bufs=(4, 2, 2)):
        self.nc, self.flags, self.waps = nc, flags, waps
        ent = stack.enter_context
        self.singles = ent(tc.tile_pool(name="singles", bufs=1))
        self.wpool = ent(tc.tile_pool(name="wpool", bufs=2))
        self.bpool = ent(tc.tile_pool(name="bpool", bufs=1))
        self.acts = ent(tc.tile_pool(name="acts", bufs=2))
        self.stats = ent(tc.tile_pool(name="stats", bufs=4))
        gpb, spb, bpb = psum_bufs
        self.gp = ent(tc.tile_pool(name="gp", bufs=gpb, space="PSUM"))
        self.sp = ent(tc.tile_pool(name="sp", bufs=spb, space="PSUM"))
        self.bp = ent(tc.tile_pool(name="bp", bufs=bpb, space="PSUM"))
        nc_ = nc
        self.ones128 = self.singles.tile([128, 1], F32, name="ones128")
        nc_.vector.memset(self.ones128, 1.0)
        self.ones1 = self.singles.tile([1, 128], F32, name="ones1")
        nc_.vector.memset(self.ones1, 1.0)
        self.eps_ln = self.singles.tile([1, 1], F32, name="eps_ln")
        nc_.vector.memset(self.eps_ln, EPS_LN)
        self.eps_rms = self.singles.tile([1, 1], F32, name="eps_rms")
        nc_.vector.memset(self.eps_rms, EPS_RMS)

    def load_w(self, nm):
        shp = {"wproj": (FC_PROJ, D), "wv": (OC, D), "wao": (OC, D),
               "wf1": (OC, DFF), "wf2": (FC_PROJ, D)}[nm]
        fc, o = shp
        tag = "w_wide" if fc * o >= FC_PROJ * D else "w_narrow"
        t = self.wpool.tile([128, fc, o], BF16, tag=tag, name=nm)
        self.nc.sync.dma_start(out=t, in_=self.waps[nm])
        return t

    def load_bias(self, nm, width):
        t = self.bpool.tile([128, width // 128], F32, tag=f"b_{nm}", name=nm)
        self.nc.scalar.dma_start(
            out=t, in_=self.waps[nm].rearrange("(c p) -> p c", p=128))
        return t

    def gemm(self, Wsb, fc, o, rhs, consume, nm):
        nc = self.nc
        for ot in range(o // 128):
            ps = self.gp.tile([128, RPC], F32, tag="gp", name=f"ps_{nm}_{ot}")
            for f in range(fc):
                nc.tensor.matmul(
                    ps, lhsT=Wsb[:, f, ot * 128:(ot + 1) * 128],
                    rhs=rhs[:, f, :], start=(f == 0), stop=(f == fc - 1))
            consume(ot, ps)

    def colsum(self, src, nm):
        nc = self.nc
        ps = self.sp.tile([1, RPC], F32, tag="sp", name=f"ss_{nm}")
        for c in range(DC):
            nc.tensor.matmul(ps, lhsT=self.ones128, rhs=src[:, c, :],
                             start=(c == 0), stop=(c == DC - 1))
        return ps

    def bcast(self, vec, nm):
        ps = self.bp.tile([128, RPC], F32, tag="bp", name=f"bc_{nm}")
        self.nc.tensor.matmul(ps, lhsT=self.ones1, rhs=vec, start=True,
                              stop=True)
        return ps

    def rstd_from_ex2(self, ex2, eps_tile, nm):
        nc = self.nc
        sd = self.stats.tile([1, RPC], F32, tag="sd", name=f"sd_{nm}")
        nc.scalar.activation(out=sd, in_=ex2,
                             func=mybir.ActivationFunctionType.Sqrt,
                             bias=eps_tile, scale=1.0)
        rs = self.stats.tile([1, RPC], F32, tag="rs", name=f"rs_{nm}")
        nc.vector.reciprocal(out=rs, in_=sd)
        return rs

    def layernorm(self, src, outs, nm, affine=None):
        nc = self.nc
        sq = self.acts.tile([128, DC, RPC], F32, tag="sq", name=f"sq_{nm}")
        for c in range(DC):
            nc.vector.tensor_mul(out=sq[:, c, :], in0=src[:, c, :],
                                 in1=src[:, c, :])
        ps_sum = self.colsum(src, f"{nm}s")
        ps_ss = self.colsum(sq, f"{nm}q")
        m1 = self.stats.tile([1, RPC], F32, tag="m1", name=f"m1_{nm}")
        nc.scalar.mul(out=m1, in_=ps_sum, mul=1.0 / D)
        e1 = self.stats.tile([1, RPC], F32, tag="e1", name=f"e1_{nm}")
        nc.scalar.mul(out=e1, in_=ps_ss, mul=1.0 / D)
        msq = self.stats.tile([1, RPC], F32, tag="msq", name=f"msq_{nm}")
        nc.vector.tensor_mul(out=msq, in0=m1, in1=m1)
        var = self.stats.tile([1, RPC], F32, tag="var", name=f"var_{nm}")
        nc.vector.tensor_sub(out=var, in0=e1, in1=msq)
        rs = self.rstd_from_ex2(var, self.eps_ln, nm)
        mb_ps = self.bcast(m1, f"{nm}m")
        rb_ps = self.bcast(rs, f"{nm}r")
        mb = self.acts.tile([128, RPC], F32, tag="mb", name=f"mb_{nm}")
        nc.vector.tensor_copy(out=mb, in_=mb_ps)
        rb = self.acts.tile([128, RPC], F32, tag="rb", name=f"rb_{nm}")
        nc.vector.tensor_copy(out=rb, in_=rb_ps)
        for c in range(DC):
            cen = sq[:, c, :]
            nc.vector.tensor_sub(out=cen, in0=src[:, c, :], in1=mb)
            if affine is None:
                for out_t in outs:
                    nc.vector.tensor_mul(out=out_t[:, c, :], in0=cen, in1=rb)
            else:
                w_t, b_t = affine
                nc.vector.tensor_mul(out=cen, in0=cen, in1=rb)
                nc.vector.tensor_scalar(
                    out=outs[0][:, c, :], in0=cen,
                    scalar1=w_t[:, c:c + 1], scalar2=b_t[:, c:c + 1],
                    op0=mybir.AluOpType.mult, op1=mybir.AluOpType.add)
                for out_t in outs[1:]:
                    nc.scalar.copy(out=out_t[:, c, :], in_=outs[0][:, c, :])

    def layer_body(self, mrg_k, h_out_dram):
        """One transformer layer: mrg_k [128, FC_PROJ, RPC] bf16 -> h_bf,
        DMA'd to h_out_dram [DC, 128, RPC]."""
        nc, flags = self.nc, self.flags
        acts = self.acts
        Wp = self.load_w("wproj")
        Wv = self.load_w("wv")
        Wa = self.load_w("wao")
        W1 = self.load_w("wf1")
        W2 = self.load_w("wf2")
        pb = self.load_bias("pbias", D) if flags.get("pbias") else None
        vb = self.load_bias("vbias", D) if flags.get("vbias") else None
        ab = self.load_bias("aobias", D) if flags.get("aobias") else None
        f1b = self.load_bias("f1bias", DFF) if flags.get("f1bias") else None
        f2b = self.load_bias("f2bias", D) if flags.get("f2bias") else None
        ln1a = ((self.load_bias("ln1w", D), self.load_bias("ln1b", D))
                if flags.get("ln1") else None)
        ln2a = ((self.load_bias("ln2w", D), self.load_bias("ln2b", D))
                if flags.get("ln2") else None)

        p_f32 = acts.tile([128, DC, RPC], F32, tag="p_f32", name="p_f32")
        p_bf = acts.tile([128, DC, RPC], BF16, tag="p_bf", name="p_bf")

        def c_proj(ot, ps):
            if pb is None:
                nc.vector.tensor_copy(out=p_f32[:, ot, :], in_=ps)
            else:
                nc.vector.tensor_scalar_add(
                    out=p_f32[:, ot, :], in0=ps, scalar1=pb[:, ot:ot + 1])
            nc.vector.tensor_copy(out=p_bf[:, ot, :], in_=p_f32[:, ot, :])
        self.gemm(Wp, FC_PROJ, D, mrg_k, c_proj, "proj")

        v_bf = acts.tile([128, DC, RPC], BF16, tag="v_bf", name="v_bf")

        def c_v(ot, ps):
            if vb is None:
                nc.vector.tensor_copy(out=v_bf[:, ot, :], in_=ps)
            else:
                nc.vector.tensor_scalar_add(
                    out=v_bf[:, ot, :], in0=ps, scalar1=vb[:, ot:ot + 1])
        self.gemm(Wv, OC, D, p_bf, c_v, "v")

        r1 = acts.tile([128, DC, RPC], F32, tag="r1", name="r1")

        def c_sa(ot, ps):
            nc.vector.tensor_add(out=r1[:, ot, :], in0=ps, in1=p_f32[:, ot, :])
            if ab is not None:
                nc.vector.tensor_scalar_add(
                    out=r1[:, ot, :], in0=r1[:, ot, :],
                    scalar1=ab[:, ot:ot + 1])
        self.gemm(Wa, OC, D, v_bf, c_sa, "sa")

        s1_f32 = acts.tile([128, DC, RPC], F32, tag="s1_f32", name="s1_f32")
        s1_bf = acts.tile([128, DC, RPC], BF16, tag="s1_bf", name="s1_bf")
        self.layernorm(r1, [s1_f32, s1_bf], "ln1", affine=ln1a)

        ff_bf = acts.tile([128, FC_PROJ, RPC], BF16, tag="ff_bf", name="ff_bf")

        def c_ff1(ot, ps):
            if f1b is None:
                nc.vector.tensor_relu(out=ff_bf[:, ot, :], in_=ps)
            else:
                nc.scalar.activation(
                    out=ff_bf[:, ot, :], in_=ps,
                    func=mybir.ActivationFunctionType.Relu,
                    bias=f1b[:, ot:ot + 1], scale=1.0)
        self.gemm(W1, OC, DFF, s1_bf, c_ff1, "ff1")

        r2 = acts.tile([128, DC, RPC], F32, tag="r2", name="r2")

        def c_ff2(ot, ps):
            nc.vector.tensor_add(out=r2[:, ot, :], in0=ps,
                                 in1=s1_f32[:, ot, :])
            if f2b is not None:
                nc.vector.tensor_scalar_add(
                    out=r2[:, ot, :], in0=r2[:, ot, :],
                    scalar1=f2b[:, ot:ot + 1])
        self.gemm(W2, FC_PROJ, D, ff_bf, c_ff2, "ff2")

        h_bf = acts.tile([128, DC, RPC], BF16, tag="h_bf", name="h_bf")
        self.layernorm(r2, [h_bf], "ln2", affine=ln2a)
        nc.scalar.dma_start(out=h_out_dram.rearrange("c p r -> p c r"),
                            in_=h_bf)

    def mrg_from_h(self, h_own_dram, me_dram):
        """mrg = [rms(h_own), rms-prepped me] — me arrives pre-normalized."""
        nc = self.nc
        acts = self.acts
        mrg = acts.tile([128, FC_PROJ, RPC], BF16, tag="mrg", name="mrg")
        nc.scalar.dma_start(out=mrg[:, DC:, :], in_=me_dram)
        hov = acts.tile([128, DC, RPC], BF16, tag="hov", name="hov")
        nc.scalar.dma_start(out=hov,
                            in_=h_own_dram.rearrange("c p r -> p c r"))
        sqh = acts.tile([128, DC, RPC], F32, tag="sq", name="sqh")
        for c in range(DC):
            nc.vector.tensor_mul(out=sqh[:, c, :], in0=hov[:, c, :],
                                 in1=hov[:, c, :])
        ps_ss = self.colsum(sqh, "rms")
        e1 = self.stats.tile([1, RPC], F32, tag="e1", name="e1_rms")
        nc.scalar.mul(out=e1, in_=ps_ss, mul=1.0 / D)
        rs = self.rstd_from_ex2(e1, self.eps_rms, "rms")
        rb_ps = self.bcast(rs, "rms")
        rb = acts.tile([128, RPC], F32, tag="rb", name="rb_rms")
        nc.vector.tensor_copy(out=rb, in_=rb_ps)
        for c in range(DC):
            nc.vector.tensor_mul(out=mrg[:, c, :], in0=hov[:, c, :], in1=rb)
        return mrg


# ========================================================== launch 1: L0 ====

def build_l0(flags, repeat=1):
    nc = bacc.Bacc("TRN2", target_bir_lowering=False, debug=False,
                   num_devices=NCORES)
    mh0 = nc.dram_tensor("mh0", [128, DC, RPC], BF16,
                         kind="ExternalInput").ap()
    me0 = nc.dram_tensor("me0", [128, DC, RPC], BF16,
                         kind="ExternalInput").ap()
    waps = declare_layer_inputs(nc, flags)
    h_out = nc.dram_tensor("h", [DC, 128, RPC], BF16,
                           kind="ExternalOutput").ap()

    from contextlib import ExitStack
    with tile.TileContext(nc) as tc, ExitStack() as stack:
        lo = LayerOps(nc, tc, flags, waps, stack)

        def body(iv=None):
            mrg = lo.acts.tile([128, FC_PROJ, RPC], BF16, tag="mrg",
                               name="mrg0")
            nc.scalar.dma_start(out=mrg[:, :DC, :], in_=mh0)
            nc.scalar.dma_start(out=mrg[:, DC:, :], in_=me0)
            lo.layer_body(mrg, h_out)

        if repeat > 1:
            with tc.For_i(0, repeat, 1):
                body()
        else:
            body()
    nc.compile()
    return nc


# ============================================== launches 2-4: [Lk] + B(k-1) =

def build_lb(with_layer, flags=None, repeat=1):
    """Unembed of the gathered previous h; optionally fused with the next
    transformer layer (whose weights stream under the unembed matmuls)."""
    nc = bacc.Bacc("TRN2", target_bir_lowering=False, debug=False,
                   num_devices=NCORES)
    hT = nc.dram_tensor("hT", [DC, 128, ROWSG], BF16,
                        kind="ExternalInput").ap()
    emb = nc.dram_tensor("emb", [128, NBLK, DC, 512], BF16,
                         kind="ExternalInput").ap()
    out = nc.dram_tensor("out", [NBLK, 128, NRT, 512], BF16,
                         kind="ExternalOutput").ap()
    if with_layer:
        h_own = nc.dram_tensor("h_own", [DC, 128, RPC], BF16,
                               kind="ExternalInput").ap()
        me_k = nc.dram_tensor("me_k", [128, DC, RPC], BF16,
                              kind="ExternalInput").ap()
        waps = declare_layer_inputs(nc, flags)
        h_out = nc.dram_tensor("h", [DC, 128, RPC], BF16,
                               kind="ExternalOutput").ap()

    from contextlib import ExitStack
    with tile.TileContext(nc) as tc, ExitStack() as stack:
        if True:
            hpool = stack.enter_context(tc.tile_pool(name="hpool", bufs=2))
            epool = stack.enter_context(tc.tile_pool(name="epool", bufs=4))
            spool = stack.enter_context(tc.tile_pool(name="spool", bufs=3))
            pp = stack.enter_context(
                tc.tile_pool(name="pp", bufs=4, space="PSUM"))
            lo = (LayerOps(nc, tc, flags, waps, stack,
                           psum_bufs=(2, 1, 1))
                  if with_layer else None)

            def body(iv=None):
                hsb = []
                for c in range(DC):
                    t = hpool.tile([128, ROWSG], BF16, tag=f"h_{c}",
                                   name=f"h_{c}")
                    nc.scalar.dma_start(out=t, in_=hT[c])
                    hsb.append(t)

                if with_layer:
                    mrg = lo.mrg_from_h(h_own, me_k)
                    lo.layer_body(mrg, h_out)

                for j, (v0, vn) in enumerate(V_BLOCKS):
                    ej = epool.tile([128, DC, 512], BF16, name="ej", tag="ej")
                    nc.scalar.dma_start(out=ej, in_=emb[:, j])
                    stb = spool.tile([128, NRT, 512], BF16, name="stb",
                                     tag="st")
                    for rt in range(NRT):
                        ps = pp.tile([128, 512], F32, name="ps", tag="ps")
                        for c in range(DC):
                            nc.tensor.matmul(
                                ps[:, :vn],
                                lhsT=hsb[c][:, rt * 128:(rt + 1) * 128],
                                rhs=ej[:, c, :vn],
                                start=(c == 0), stop=(c == DC - 1))
                        if rt % 2 == 0:
                            nc.vector.tensor_copy(out=stb[:, rt, :vn],
                                                  in_=ps[:, :vn])
                        else:
                            nc.scalar.copy(out=stb[:, rt, :vn],
                                           in_=ps[:, :vn])
                    eng = nc.sync if j % 2 else nc.scalar
                    eng.dma_start(out=out[j], in_=stb)

            if repeat > 1:
                with tc.For_i(0, repeat, 1):
                    body()
            else:
                body()
    nc.compile()
    return nc


# ================================================================ runner ====

def make_runner(nc, n_cores=NCORES):
    import jax
    from jax.sharding import Mesh, PartitionSpec
    from jax.experimental.shard_map import shard_map

    bass2jax.install_neuronx_cc_hook()
    partition_name = (nc.partition_id_tensor.name
                      if nc.partition_id_tensor else None)
    in_names, out_names, out_avals, zero_outs = [], [], [], []
    for alloc in nc.m.functions[0].allocations:
        if not isinstance(alloc, mybir.MemoryLocationSet):
            continue
        name = alloc.memorylocations[0].name
        if alloc.kind == "ExternalInput":
            if name != partition_name:
                in_names.append(name)
        elif alloc.kind == "ExternalOutput":
            out_names.append(name)
            shape = tuple(alloc.tensor_shape)
            dtype = mybir.dt.np(alloc.dtype)
            out_avals.append(jax.core.ShapedArray(shape, dtype))
            zero_outs.append(np.zeros(shape, dtype))
    n_params = len(in_names)
    in_names_all = in_names + out_names
    if partition_name is not None:
        in_names_all.append(partition_name)

    def _body(*args):
        operands = list(args)
        if partition_name is not None:
            operands.append(bass2jax.partition_id_tensor())
        outs = bass2jax._bass_exec_p.bind(
            *operands, out_avals=tuple(out_avals),
            in_names=tuple(in_names_all),
            out_names=tuple(out_names), lowering_input_output_aliases=(),
            sim_require_finite=True, sim_require_nnan=True, nc=nc)
        return tuple(outs)

    devices = jax.devices()[:n_cores]
    mesh = Mesh(np.asarray(devices), ("core",))
    in_specs = (PartitionSpec("core"),) * (n_params + len(out_names))
    out_specs = (PartitionSpec("core"),) * len(out_names)
    fn = jax.jit(
        shard_map(_body, mesh=mesh, in_specs=in_specs, out_specs=out_specs,
                  check_rep=False),
        keep_unused=True)

    def stage(in_maps):
        from jax.sharding import NamedSharding
        sh = NamedSharding(mesh, PartitionSpec("core"))
        concat_in = [np.concatenate([np.asarray(m[name]) for m in in_maps],
                                    axis=0) for name in in_names]
        concat_zeros = [np.zeros((n_cores * z.shape[0], *z.shape[1:]), z.dtype)
                        for z in zero_outs]
        return [jax.device_put(a, sh) for a in concat_in + concat_zeros]

    def run_staged(args):
        out_arrs = fn(*args)
        return [
            {name:
             np.asarray(out_arrs[i]).reshape(n_cores, *out_avals[i].shape)[c]
             for i, name in enumerate(out_names)}
            for c in range(n_cores)
        ]

    def run(in_maps):
        return run_staged(stage(in_maps))

    class Runner:
        pass

    r = Runner()
    r.run = run
    r.stage = stage
    r.run_staged = run_staged
    r.fn = fn
    return r


# ============================================================= host prep ====

def _t_layout(a):
    """[rows, D] → [128, DC, rows] (partition-major T-layout)."""
    return np.ascontiguousarray(
        a.T.reshape(DC, 128, a.shape[0]).transpose(1, 0, 2))


def prep_stage_a(inputs, flags):
    x = np.asarray(inputs["x"], np.float32)
    rms_w = np.asarray(inputs["rms_w"], np.float32)

    def rms_nw(v):  # rms without the learned scale (folded into wproj)
        return v / np.sqrt((v * v).mean(-1, keepdims=True) + EPS_RMS)

    xr = x[:, :P, :].reshape(ROWS, D)
    mh0_full = rms_nw(xr)
    me_full = np.stack([
        rms_nw(x[:, kk + 1:kk + 1 + P, :].reshape(ROWS, D))
        for kk in range(K)])

    mh0_cores, me_cores = [], []
    for c in range(NCORES):
        r0 = c * RPC
        n = max(0, min(RPC, ROWS - r0))
        sl = np.zeros((RPC, D), np.float32)
        sl[:n] = mh0_full[r0:r0 + n]
        mh0_cores.append(_t_layout(sl).astype(BF16_NP))
        mes = []
        for kk in range(K):
            sk = np.zeros((RPC, D), np.float32)
            sk[:n] = me_full[kk, r0:r0 + n]
            mes.append(_t_layout(sk))
        me_cores.append(np.stack(mes).astype(BF16_NP))

    proj_w = np.asarray(inputs["proj_w"], np.float32)
    # fold rms_w into the proj weight's input rows (exact)
    scale = np.concatenate([rms_w, rms_w])            # [2D]
    wproj = proj_w.transpose(0, 2, 1) * scale[None, :, None]
    attn_in_w = np.asarray(inputs["attn_in_w"], np.float32)

    def w_tiled(wt):
        """[K, F, O] transposed weight → pre-tiled [K, 128, FC, O] bf16."""
        Kk, F, O = wt.shape
        return np.ascontiguousarray(
            wt.reshape(Kk, F // 128, 128, O).transpose(0, 2, 1, 3)
        ).astype(BF16_NP)

    weights = {
        "wproj": w_tiled(wproj),
        "wv": w_tiled(attn_in_w[:, 2 * D:, :].transpose(0, 2, 1)),
        "wao": w_tiled(np.asarray(
            inputs["attn_out_w"], np.float32).transpose(0, 2, 1)),
        "wf1": w_tiled(np.asarray(
            inputs["ff1_w"], np.float32).transpose(0, 2, 1)),
        "wf2": w_tiled(np.asarray(
            inputs["ff2_w"], np.float32).transpose(0, 2, 1)),
    }
    if flags.get("pbias"):
        weights["pbias"] = np.asarray(inputs["proj_b"], np.float32)
    if flags.get("vbias"):
        weights["vbias"] = np.ascontiguousarray(
            np.asarray(inputs["attn_in_b"], np.float32)[:, 2 * D:])
    if flags.get("aobias"):
        weights["aobias"] = np.asarray(inputs["attn_out_b"], np.float32)
    if flags.get("f1bias"):
        weights["f1bias"] = np.asarray(inputs["ff1_b"], np.float32)
    if flags.get("f2bias"):
        weights["f2bias"] = np.asarray(inputs["ff2_b"], np.float32)
    if flags.get("ln1"):
        weights["ln1w"] = np.asarray(inputs["ln1_w"], np.float32)
        weights["ln1b"] = np.asarray(inputs["ln1_b"], np.float32)
    if flags.get("ln2"):
        weights["ln2w"] = np.asarray(inputs["ln2_w"], np.float32)
        weights["ln2b"] = np.asarray(inputs["ln2_b"], np.float32)
    return mh0_cores, me_cores, weights


def layer_weights(weights, flags, k):
    w = {nm: weights[nm][k] for nm in ("wproj", "wv", "wao", "wf1", "wf2")}
    for nm in ("pbias", "vbias", "aobias", "f1bias", "f2bias",
               "ln1w", "ln1b", "ln2w", "ln2b"):
        if nm in weights:
            w[nm] = weights[nm][k]
    return w


def prep_stage_b_emb(embed_w):
    """→ per-core [128, NBLK, DC, 512] bf16, contiguous per partition-row."""
    VB = NBLK * 512
    embT = np.zeros((D, NCORES * VB), dtype=BF16_NP)
    et = np.asarray(embed_w, np.float32).T.astype(BF16_NP)   # [D, V]
    for s in range(NCORES):
        n = min(VS, max(0, V - s * VS))
        embT[:, s * VB:s * VB + n] = et[:, s * VS:s * VS + n]
    shards = []
    for s in range(NCORES):
        sh = embT[:, s * VB:(s + 1) * VB]                    # [D, NBLK*512]
        shards.append(np.ascontiguousarray(
            sh.reshape(DC, 128, NBLK, 512).transpose(1, 2, 0, 3)))
    return shards


_CACHE = {}


def gather_h(res):
    """per-core h [DC,128,RPC] → hT [DC,128,ROWSG]."""
    return np.concatenate([r["h"] for r in res], axis=2)


def unscramble_out(res_list):
    """per-core out [NBLK,128,NRT,512] bf16 → [ROWS, V] f32 (one k)."""
    parts = []
    for r in res_list:
        ob = r["out"]
        f = ob.transpose(2, 1, 0, 3).reshape(NRT * 128, NBLK * 512)
        parts.append(f[:ROWS, :VS])
    return np.concatenate(parts, axis=1)[:, :V].astype(np.float32)


def kernel(**inputs):
    flags = {
        "pbias": np.any(np.asarray(inputs["proj_b"])),
        "vbias": np.any(np.asarray(inputs["attn_in_b"])[:, 2 * D:]),
        "aobias": np.any(np.asarray(inputs["attn_out_b"])),
        "f1bias": np.any(np.asarray(inputs["ff1_b"])),
        "f2bias": np.any(np.asarray(inputs["ff2_b"])),
        "ln1": (np.any(np.asarray(inputs["ln1_b"]))
                or not np.all(np.asarray(inputs["ln1_w"]) == 1.0)),
        "ln2": (np.any(np.asarray(inputs["ln2_b"]))
                or not np.all(np.asarray(inputs["ln2_w"]) == 1.0)),
    }
    flags = {k: bool(v) for k, v in flags.items()}
    fkey = tuple(sorted(flags.items()))

    if ("l0", fkey) not in _CACHE:
        _CACHE[("l0", fkey)] = make_runner(build_l0(flags))
    if ("lb", fkey) not in _CACHE:
        _CACHE[("lb", fkey)] = make_runner(build_lb(True, flags))
    if "b" not in _CACHE:
        _CACHE["b"] = make_runner(build_lb(False))
    run_l0 = _CACHE[("l0", fkey)]
    run_lb = _CACHE[("lb", fkey)]
    run_b = _CACHE["b"]

    mh0_cores, me_cores, weights = prep_stage_a(inputs, flags)
    emb_shards = prep_stage_b_emb(inputs["embed_w"])

    # launch 1: layer 0
    w0 = layer_weights(weights, flags, 0)
    res1 = run_l0.run([dict(mh0=mh0_cores[c], me0=me_cores[c][0], **w0)
                       for c in range(NCORES)])
    hT0 = gather_h(res1)

    # launch 2: layer 1 + unembed(h0)
    w1 = layer_weights(weights, flags, 1)
    res2 = run_lb.run([dict(hT=hT0, emb=emb_shards[c], h_own=res1[c]["h"],
                            me_k=me_cores[c][1], **w1)
                       for c in range(NCORES)])
    hT1 = gather_h(res2)
    log0 = unscramble_out(res2)

    # launch 3: layer 2 + unembed(h1)
    w2 = layer_weights(weights, flags, 2)
    res3 = run_lb.run([dict(hT=hT1, emb=emb_shards[c], h_own=res2[c]["h"],
                            me_k=me_cores[c][2], **w2)
                       for c in range(NCORES)])
    hT2 = gather_h(res3)
    log1 = unscramble_out(res3)

    # launch 4: unembed(h2)
    res4 = run_b.run([dict(hT=hT2, emb=emb_shards[c])
                      for c in range(NCORES)])
    log2 = unscramble_out(res4)

    full = np.stack([log0, log1, log2], axis=1)      # [ROWS, K, V]
    return np.ascontiguousarray(full).reshape(B, P, K, V)
